# revision 2
# baseline (speedup 1.0000x reference)
"""DeepFilter (deep filtering) Trainium2 kernel.

Full-input contract: kernel(spec, coefs) -> out, all full-shape numpy arrays.
Sharding: pure data-parallel over the batch dim (8 batches -> 8 cores).

Per-core computation (B=1 slice):
  out[c, t, f<256] = sum_k complex( spec[:, t+k-4, f] * coefs[k-tap, t, f] )
  out[c, t, f>=256] = spec[c, t, f]   (passthrough)

Implementation notes:
  - T tiles of 124 output rows; the product tile spans spec rows
    [t0-4, t0+124) so every tap k reads product partitions [k, 124+k).
  - Coef tap-plane k is DMA-loaded with row offset t0-k, aligning
    c_k[t'+4-k] with spec[t'] in the same partition.
  - DVE computes 4 products (the -pi*ci sign is fused via
    scalar_tensor_tensor), GPSIMD combines them into real/imag planes,
    and the TensorEngine applies 5 accumulating fp32 matmuls with 0/1
    shift matrices (exact on HW) to do the cross-partition tap-shift-sum.
"""

import numpy as np

import concourse.bass as bass
import concourse.mybir as mybir
import concourse.tile as tile
from concourse.bass_types import AP
from concourse.bass_utils import run_bass_kernel_spmd

F32 = mybir.dt.float32

B, T, F_TOTAL = 8, 4096, 481
NF = 256          # filtered freqs
FP = F_TOTAL - NF  # passthrough freqs (225)
K = 5             # taps
TS = 124          # output rows per tile
PAD = 4           # frame_size - 1 - lookahead

# ---------------------------------------------------------------------------
# Workaround for this container's walrus: at most ONE sync-wait per
# instruction. Rewrite the BIR JSON, splitting extra waits onto preceding
# same-engine EventSemaphore carriers.
# ---------------------------------------------------------------------------


def _split_bir_waits(bir_bytes: bytes) -> bytes:
    import orjson

    d = orjson.loads(bir_bytes)
    n = 0
    for fn in d.get("functions", []):
        for bb in fn.get("blocks", []):
            out = []
            for ins in bb.get("instructions", []):
                si = ins.get("sync_info")
                if si and len(si.get("on_wait") or []) > 1:
                    waits = si["on_wait"]
                    for w in waits[:-1]:
                        n += 1
                        out.append(
                            {
                                "debug": ins.get("debug"),
                                "engine": ins["engine"],
                                "ins": [],
                                "name": f"antwaitsplit_{n}",
                                "opcode": "EventSemaphore",
                                "outs": [],
                                "sync_info": {"on_update": [], "on_wait": [w]},
                            }
                        )
                    si["on_wait"] = [waits[-1]]
                out.append(ins)
            bb["instructions"] = out
    return orjson.dumps(d)


def _install_patches():
    import concourse.bass2jax as bass2jax

    if getattr(bass2jax, "_ant_wait_split_installed", False):
        return
    orig = bass2jax._decompress_ant_bir

    def wrapped(v):
        return _split_bir_waits(orig(v))

    bass2jax._decompress_ant_bir = wrapped
    bass2jax._ant_wait_split_installed = True


# ---------------------------------------------------------------------------
# Kernel build
# ---------------------------------------------------------------------------


def _ap(t, offset, dims):
    """Raw access pattern on a DRAM tensor: dims = [[step, count], ...] in
    elements."""
    return AP(t, offset, [list(d) for d in dims])


def _build_nc():
    nc = bass.Bass()
    spec = nc.dram_tensor("spec", [2, T, F_TOTAL], F32, kind="ExternalInput")
    coefs = nc.dram_tensor("coefs", [2 * K, T, NF], F32, kind="ExternalInput")
    out = nc.dram_tensor("out", [2, T, F_TOTAL], F32, kind="ExternalOutput")

    n_tiles = (T - TS) // TS + 1  # 33 uniform tiles ...
    tile_starts = [TS * i for i in range(n_tiles)]
    if tile_starts[-1] + TS < T:
        tile_starts.append(T - TS)  # ... + one overlapping tail tile

    with tile.TileContext(nc) as tc:
        with (
            tc.tile_pool(name="const", bufs=1) as cpool,
            tc.tile_pool(name="io", bufs=3) as iop,
            tc.tile_pool(name="prod", bufs=2) as pp,
            tc.tile_pool(name="psum", bufs=2, space="PSUM") as psp,
        ):
            # Shift matrices: IBIG[p, cc] = 1.0 iff p == cc - 4.
            # lhsT for tap k = IBIG[:, 4+k : 128+k]  (S_k[p, m] = [p == m+k])
            ones = cpool.tile([128, 132], F32, tag="ones")
            ibig = cpool.tile([128, 132], F32, tag="ibig")
            nc.vector.memset(ones[:], 1.0)
            nc.gpsimd.affine_select(
                ibig[:],
                ones[:],
                pattern=[[-1, 132]],
                compare_op=mybir.AluOpType.is_equal,
                fill=0.0,
                base=PAD,
                channel_multiplier=1,
            )

            for t0 in tile_starts:
                rs = t0 - PAD  # first spec row of the product tile
                # ---- load spec rows [rs, rs+128) as [t, c, F_TOTAL] ----
                S = iop.tile([128, 2, F_TOTAL], F32, tag="S")
                if rs < 0:
                    nc.gpsimd.memset(S[0:-rs, :, :], 0.0)
                    nc.sync.dma_start(
                        S[-rs:128, :, :],
                        _ap(spec, 0, [[F_TOTAL, 128 + rs], [T * F_TOTAL, 2], [1, F_TOTAL]]),
                    )
                else:
                    nc.sync.dma_start(
                        S[:],
                        _ap(spec, rs * F_TOTAL, [[F_TOTAL, 128], [T * F_TOTAL, 2], [1, F_TOTAL]]),
                    )

                # ---- load coefs as [t, k, c, NF], tap k shifted by -k ----
                CC = iop.tile([128, K, 2, NF], F32, tag="CC")
                lo = t0 - (K - 1)   # lowest source row used (tap k=4)
                hi = t0 + 128      # one past highest source row (tap k=0)
                if lo < 0 or hi > T:
                    nc.gpsimd.memset(CC[:], 0.0)
                    for c in range(2):
                        for k in range(K):
                            r0, r1 = t0 - k, t0 + 128 - k
                            p0 = max(0, -r0)
                            r0 = max(r0, 0)
                            r1 = min(r1, T)
                            nc.sync.dma_start(
                                CC[p0 : p0 + (r1 - r0), k, c, :],
                                _ap(
                                    coefs,
                                    ((c * K + k) * T + r0) * NF,
                                    [[NF, r1 - r0], [1, NF]],
                                ),
                            )
                else:
                    for c in range(2):
                        nc.sync.dma_start(
                            CC[:, :, c, :],
                            _ap(
                                coefs,
                                (c * K * T + t0) * NF,
                                [[NF, 128], [(T - 1) * NF, K], [1, NF]],
                            ),
                        )

                # ---- products (DVE) ----
                pr = S[:, 0, 0:NF].unsqueeze(1).broadcast_to([128, K, NF])
                pi = S[:, 1, 0:NF].unsqueeze(1).broadcast_to([128, K, NF])
                cr = CC[:, :, 0, :]
                ci = CC[:, :, 1, :]
                M1 = pp.tile([128, K, NF], F32, tag="M1")   # pr*cr
                M2 = pp.tile([128, K, NF], F32, tag="M2")   # -pi*ci
                M3 = pp.tile([128, K, NF], F32, tag="M3")   # pi*cr
                M4 = pp.tile([128, K, NF], F32, tag="M4")   # pr*ci
                nc.vector.tensor_tensor(M1[:], pr, cr, mybir.AluOpType.mult)
                nc.vector.scalar_tensor_tensor(
                    M2[:], pi, -1.0, ci, mybir.AluOpType.mult, mybir.AluOpType.mult
                )
                nc.vector.tensor_tensor(M3[:], pi, cr, mybir.AluOpType.mult)
                nc.vector.tensor_tensor(M4[:], pr, ci, mybir.AluOpType.mult)

                # ---- combine into [t, k, (re, im), NF] (GPSIMD) ----
                DE = pp.tile([128, K, 2, NF], F32, tag="DE")
                nc.gpsimd.tensor_tensor(
                    DE[:, :, 0, :], M1[:], M2[:], mybir.AluOpType.add
                )
                nc.gpsimd.tensor_tensor(
                    DE[:, :, 1, :], M3[:], M4[:], mybir.AluOpType.add
                )

                # ---- tap-shift-sum on PE: psum[m] = sum_k DE[m+k, k] ----
                ps = psp.tile([TS, 2 * NF], F32, tag="ps")
                for k in range(K):
                    nc.tensor.matmul(
                        ps[:],
                        ibig[:, PAD + k : PAD + k + TS],
                        DE[:, k].rearrange("p c f -> p (c f)"),
                        start=(k == 0),
                        stop=(k == K - 1),
                    )

                # ---- PSUM -> SBUF, then DMA out ----
                osb = iop.tile([TS, 2 * NF], F32, tag="osb")
                nc.scalar.copy(osb[:], ps[:])
                nc.scalar.dma_start(
                    _ap(out, t0 * F_TOTAL, [[F_TOTAL, TS], [T * F_TOTAL, 2], [1, NF]]),
                    osb[:].rearrange("p (c f) -> p c f", c=2),
                )
                nc.scalar.dma_start(
                    _ap(
                        out,
                        t0 * F_TOTAL + NF,
                        [[F_TOTAL, TS], [T * F_TOTAL, 2], [1, FP]],
                    ),
                    S[PAD : PAD + TS, :, NF:F_TOTAL],
                )
    return nc


_NC = None


def kernel(spec: np.ndarray, coefs: np.ndarray) -> np.ndarray:
    global _NC
    _install_patches()
    if _NC is None:
        _NC = _build_nc()
    spec = np.ascontiguousarray(spec, dtype=np.float32)
    coefs = np.ascontiguousarray(coefs, dtype=np.float32)
    in_maps = [
        {"spec": np.ascontiguousarray(spec[b]), "coefs": np.ascontiguousarray(coefs[b])}
        for b in range(B)
    ]
    res = run_bass_kernel_spmd(_NC, in_maps, core_ids=list(range(B)))
    return np.stack([res.results[b]["out"] for b in range(B)], axis=0)
